# revision 54
# baseline (speedup 1.0000x reference)
"""Multi-head attention (B=2, L=2048, dim=1024, 16 heads) on 8 Trainium2 cores.

Sharding: 8 cores = 2 (batch) x 4 (head groups of 4 heads). Each core runs an
identical Bass program on its own slice (SPMD, no collectives); the host sums
the 4 per-head-group partial projection outputs per batch and adds the bias.

Per-core dataflow (bf16 matmul operands, fp32 PSUM accumulation):
  Inputs land via 7 large contiguous DMA descriptors, ALL issued on the sync
  queue in need-order: SDMA engines round-robin across queues-with-work, so
  a single FIFO queue gives the first-needed bytes the full aggregate
  bandwidth instead of an equal share.
  V token-major [128 tok, 4 heads, 64+1] (ones column fused for the softmax
    denominator), qT/kT feature-major [128 (2 heads x 64d), 2048]
  Per k-block: score tile ST[k, q] for both heads (the two K=64 matmuls run
    concurrently via PE row tiling); one full-width exp per k-block,
    alternating engines (exact table exp on ScalarE / Schraudolph bit-trick
    on the DVE: bits = S*A + B as int16, bitcast to bf16, ~3% relative,
    cancels in the softmax ratio).
  OT_h[d, q] += V_h.T @ PT_h   (M=65: row 64 accumulates the denominator)
  normalization (the critical redesign vs the 261us version): pair N's chain
    is emitted step-by-step inside pair N+1's k-loop (slots kb=6..15) with a
    dummy-element gate on pair N+1's kb-6 score tile, so the Tile
    scheduler's ready-heap cannot place these long-latency ops ahead of
    pair N+1's exps in the DVE queue (head-of-line blocking there stalled
    the PE ~4us at every pair boundary). The GpSimd partition_broadcast
    (constant ~4.5us queue wait) is replaced by PE outer-products
    (ones[1,64].T @ recip row) into the unused partitions 64..127 of the OT
    PSUM banks; ScalarE stages them to SBUF (DVE ops may read only one PSUM
    operand) and the muls run on the DVE (32-wide ops for head A, 64-wide
    quadrant-pair-aligned op for head B crossing into partitions 64..127).
    Denominator rows: ScalarE copies (partition 64) + one SBUF-SBUF DMA to
    partition 0 + reciprocal_approx_fast; the last pair instead uses DVE
    cross-quadrant copies and per-head pipelined chains to shorten the
    exposed kernel tail.
  out[tok, c] = OT_norm.T @ wpT -> PSUM, staged to fp16 SBUF, DMA'd to DRAM
    on two alternating issue queues.
"""

import os
import math
import numpy as np

B, L, C = 2, 2048, 1024
H, D = 16, 64
HL = 4            # heads per core (local)
PAIRS = 2         # head pairs per core
CT = C // 128     # 8 contraction tiles for the projections
TOK = L // 128    # 16 key-token tiles
QW = 512          # query tile width
QS = L // QW      # 4 query tiles
NCORES = 8
PEND = 3          # k-blocks of slack between a PV matmul and its exp

# Schraudolph exp constants (bf16 bit pattern): bits = S_raw * EXPA + EXPB
EXPA = 0.125 * math.log2(math.e) * 128.0
EXPB = 128.0 * (127.0 - 0.043035)
KEEPWARM = False
LASTPAIR_DVE_DEN = True   # DVE cross-quadrant copy of den rows for last pair

_cache = {}


def _build_nc():
    import concourse.bass as bass
    import concourse.mybir as mybir
    import concourse.tile as tile
    from concourse import bacc

    F32 = mybir.dt.float32
    BF16 = mybir.dt.bfloat16
    F16 = mybir.dt.float16
    I16 = mybir.dt.int16
    EXP = mybir.ActivationFunctionType.Exp
    MUL = mybir.AluOpType.mult
    ADD = mybir.AluOpType.add

    nc = bacc.Bacc("TRN2", target_bir_lowering=False, debug=False,
                   num_devices=NCORES)

    # channel-tile-major 3D layouts prepared on the host so each load is one
    # big strided descriptor
    xTb = [nc.declare_dram_parameter(f"xTb{i}", [128, CT, QW], BF16,
                                       isOutput=False) for i in range(1, 4)]
    xTb0 = [nc.declare_dram_parameter(f"xTb0{h}", [128, CT // 2, QW], BF16,
                                      isOutput=False) for h in ("a", "b")]
    wqkT = nc.declare_dram_parameter("wqkT", [128, CT, 2 * HL * D], BF16,
                                     isOutput=False)
    wvT = nc.declare_dram_parameter("wvT", [128, CT, HL * D], BF16,
                                    isOutput=False)
    wpT = nc.declare_dram_parameter("wpT", [128, PAIRS, C], BF16,
                                    isOutput=False)
    out = nc.declare_dram_parameter("out", [L, C], F16, isOutput=True)

    with tile.TileContext(nc) as tc:
        from contextlib import ExitStack
        with ExitStack() as ctx:
            qkpool = ctx.enter_context(tc.tile_pool(name="qk", bufs=1))
            vpool = ctx.enter_context(tc.tile_pool(name="v", bufs=1))
            wppool = ctx.enter_context(tc.tile_pool(name="wp", bufs=1))
            # per-head score tiles: 2 bufs per head tag = 4 PSUM banks
            psS = ctx.enter_context(tc.tile_pool(name="psS", bufs=2, space="PSUM"))
            phase1 = ExitStack()
            xpool = phase1.enter_context(tc.tile_pool(name="x", bufs=1))
            wpool = phase1.enter_context(tc.tile_pool(name="w", bufs=1))
            psA = phase1.enter_context(tc.tile_pool(name="psA", bufs=2, space="PSUM"))

            # ---- input loads: 7 large descriptors spread over 3 queues.
            # x is split into 4 column blocks (one tile per block so the
            # tile-granular DMA dependency gates only its own consumers).
            x_b = [None] + [xpool.tile([128, CT, QW], BF16,
                                       name=f"xb{i}", tag=f"xb{i}")
                            for i in range(1, 4)]
            x_b0 = [xpool.tile([128, CT // 2, QW], BF16,
                               name=f"xb0{h}", tag=f"xb0{h}")
                    for h in range(2)]

            def xs(blk, c):
                # channel-tile c of x column block blk (block 0 is split in
                # two c-halves so the first compute gates on half the bytes)
                if blk == 0:
                    return x_b0[c // 4][:, c % 4, :]
                return x_b[blk][:, c, :]
            wv_sb = wpool.tile([128, CT, HL * D], BF16, name="wv", tag="wv")
            wqk_sb = wpool.tile([128, CT, 2 * HL * D], BF16, name="wqk", tag="wqk")
            wp_all = wppool.tile([128, PAIRS, C], BF16, name="wp", tag="wp")

            # ALL input loads on one queue, FIFO in need-order: with a single
            # queue holding DMA work, every SDMA engine drains it exclusively,
            # so the first-needed bytes get the full aggregate bandwidth
            # (spreading across issue queues gives each an equal share and
            # starves the critical first loads behind non-critical ones)
            nc.sync.dma_start(out=wv_sb, in_=wvT[:, :, :])
            nc.sync.dma_start(out=x_b0[0], in_=xTb0[0][:, :, :])
            nc.sync.dma_start(out=x_b0[1], in_=xTb0[1][:, :, :])
            nc.sync.dma_start(out=wqk_sb, in_=wqkT[:, :, :])
            nc.sync.dma_start(out=x_b[1], in_=xTb[0][:, :, :])
            nc.sync.dma_start(out=x_b[2], in_=xTb[1][:, :, :])
            nc.sync.dma_start(out=x_b[3], in_=xTb[2][:, :, :])
            nc.sync.dma_start(out=wp_all, in_=wpT[:, :, :])

            # ---- V token-major: v[t] = [128 tok, HL, D+1] (ones col fused) --
            ones_s = vpool.tile([128, HL, 1], F32, name="ones_s", tag="ones_s")
            nc.vector.memset(ones_s, 1.0)
            # [1, 64] ones row: stationary operand of the PE outer-product
            # that broadcasts the softmax reciprocals across partitions
            ones_r = vpool.tile([1, 64], F32, name="ones_r", tag="ones_r")
            nc.vector.memset(ones_r, 1.0)
            ones_rh = vpool.tile([1, 64], BF16, name="ones_rh", tag="ones_rh")
            nc.vector.memset(ones_rh, 1.0)
            v_t = [vpool.tile([128, HL, D + 1], BF16, name=f"v{t}", tag=f"v{t}")
                   for t in range(TOK)]

            def vchunk(t):
                ps = psA.tile([128, HL * D], F32, name="psv", tag="ps")
                tc4 = t % 4
                for c in range(CT):
                    nc.tensor.matmul(
                        ps,
                        lhsT=xs(t // 4, c)[:, 128 * tc4:128 * (tc4 + 1)],
                        rhs=wv_sb[:, c, :],
                        start=(c == 0), stop=(c == CT - 1),
                    )
                vt = v_t[t]
                nc.vector.tensor_copy(out=vt[:, :, D:D + 1], in_=ones_s)
                nc.vector.tensor_copy(
                    out=vt[:, :, 0:D],
                    in_=ps.rearrange("p (h d) -> p h d", h=HL),
                )

            # ---- Q/K feature-major per pair: [128 (2h x 64d), L] ------------
            qk_t = {}
            for p in range(PAIRS):
                for nm in ("q", "k"):
                    qk_t[(nm, p)] = qkpool.tile(
                        [128, L], BF16, name=f"{nm}{p}", tag=f"{nm}{p}")

            qk_evac = [0]

            def qkchunk(nm, p, ns):
                j = 0 if nm == "q" else 1
                ps = psA.tile([128, QW], F32, name="psqk", tag="ps")
                for c in range(CT):
                    nc.tensor.matmul(
                        ps,
                        lhsT=wqk_sb[:, c, j * HL * D + 128 * p:
                                    j * HL * D + 128 * (p + 1)],
                        rhs=xs(ns, c),
                        start=(c == 0), stop=(c == CT - 1),
                    )
                dst = qk_t[(nm, p)][:, QW * ns:QW * (ns + 1)]
                # alternate PSUM evacuation between ScalarE and the DVE
                if qk_evac[0] % 2 == 0:
                    nc.scalar.copy(out=dst, in_=ps)
                else:
                    nc.vector.tensor_copy(out=dst, in_=ps)
                qk_evac[0] += 1

            # emission grouped by which x column block each matmul needs
            for t in range(4):
                vchunk(t)
            qkchunk("k", 0, 0), qkchunk("k", 1, 0)
            for t in range(4, 8):
                vchunk(t)
            qkchunk("k", 0, 1), qkchunk("k", 1, 1)
            qkchunk("q", 0, 0), qkchunk("q", 1, 0)
            for t in range(8, 12):
                vchunk(t)
            qkchunk("k", 0, 2), qkchunk("k", 1, 2)
            qkchunk("q", 0, 1), qkchunk("q", 1, 1)
            for t in range(12, 16):
                vchunk(t)
            qkchunk("k", 0, 3), qkchunk("k", 1, 3)
            for ns in (2, 3):
                qkchunk("q", 0, ns), qkchunk("q", 1, ns)

            phase1.close()
            # ---- phase 2 pools (reuse the x/w SBUF + psA PSUM space) --------
            psO = ctx.enter_context(tc.tile_pool(name="psO", bufs=4, space="PSUM"))
            otpool = ctx.enter_context(tc.tile_pool(name="ot", bufs=1))
            ptpool = ctx.enter_context(tc.tile_pool(name="pt", bufs=PEND))
            rpool = ctx.enter_context(tc.tile_pool(name="r", bufs=2))
            obpool = ctx.enter_context(tc.tile_pool(name="ob", bufs=4))

            out_qs = [nc.sync, nc.gpsimd]
            _out_rr = [0]

            def proj_chunk(qs, last=False):
                # PSUM -> fp16 SBUF staging split across ScalarE/DVE, then out.
                # Output DMAs round-robin over four issue queues so the final
                # chunk's descriptor-issue time does not serialize the drain.
                for t in range(QW // 128 * qs, QW // 128 * (qs + 1)):
                    ob = obpool.tile([128, C], F16, name="ob", tag="ob")
                    for nh in range(C // QW):
                        ps = psO.tile([128, QW], F32, name="psp", tag="ot")
                        for p2 in range(PAIRS):
                            nc.tensor.matmul(
                                ps,
                                lhsT=ot_sb[p2][qs][:, 128 * (t % (QW // 128)):
                                                   128 * (t % (QW // 128) + 1)],
                                rhs=wp_all[:, p2, QW * nh:QW * (nh + 1)],
                                start=(p2 == 0), stop=(p2 == PAIRS - 1),
                            )
                        dst = ob[:, QW * nh:QW * (nh + 1)]
                        if last:
                            # engine-parallel half-copies: the final chunk's
                            # staging sits on the strictly serial endgame
                            # path and both engines are otherwise idling
                            h = QW // 2
                            nc.scalar.copy(out=dst[:, 0:h], in_=ps[:, 0:h])
                            nc.vector.tensor_copy(out=dst[:, h:QW],
                                                  in_=ps[:, h:QW])
                        elif nh == 0:
                            nc.scalar.copy(out=dst, in_=ps)
                        else:
                            nc.vector.tensor_copy(out=dst, in_=ps)
                        orow = out[128 * t:128 * (t + 1), :]
                        eng = out_qs[_out_rr[0] % len(out_qs)]
                        _out_rr[0] += 1
                        eng.dma_start(
                            out=orow[:, QW * nh:QW * (nh + 1)], in_=dst)

            # ---- attention --------------------------------------------------
            ot_sb = [[otpool.tile([128, QW], BF16, name=f"otp{p}q{q}",
                                  tag=f"otp{p}q{q}")
                      for q in range(QS)] for p in range(PAIRS)]

            def emit_norm(pp, pqs, ot_a, ot_b, fast):
                # Normalization WITHOUT GpSimd partition_broadcast (that op
                # showed a constant ~4.5us queue wait gating every pair
                # boundary). The reciprocal row is broadcast across
                # partitions by a PE outer-product (ones[1,64].T @ rsb) into
                # the unused partitions 64-127 / vacated 0-63 of the ot_a
                # PSUM bank; the muls run on the DVE reading PSUM directly.
                # fast=True (last pair): den rows hop to partition 0 via DVE
                # cross-quadrant copies; else ScalarE stages them at
                # partition 64 and one SBUF-SBUF DMA moves them to 0.
                den64 = rpool.tile([65, 2 * QW], F32, name="den64", tag="den64")
                den0 = rpool.tile([1, 2 * QW + 8], F32, name="den0", tag="den0")
                rsb = rpool.tile([1, 2 * QW], F32, name="rsb", tag="rsb")
                rsbh = rpool.tile([1, 2 * QW], BF16, name="rsbh", tag="rsbh")
                rbc = rpool.tile([128, QW], F32, name="rbc", tag="rbc")
                dst = ot_sb[pp][pqs]

                def s_dummy(st):
                    # real dependency on the NEXT pair's k-block: keeps the
                    # chain's DVE ops out of the queue until mid-pair
                    nc.vector.tensor_copy(
                        out=den0[0:1, 2 * QW:2 * QW + 1], in_=st[0:1, 0:1])

                def s_den_a(st):
                    if fast:
                        nc.vector.tensor_copy(out=den0[0:1, 0:QW],
                                              in_=ot_a[64:65, :])
                    else:
                        nc.scalar.copy(out=den64[64:65, 0:QW],
                                       in_=ot_a[64:65, :])

                def s_den_b(st):
                    if fast:
                        nc.vector.tensor_copy(out=den0[0:1, QW:2 * QW],
                                              in_=ot_b[64:65, :])
                    else:
                        nc.scalar.copy(out=den64[64:65, QW:2 * QW],
                                       in_=ot_b[64:65, :])

                def s_dma(st):
                    if not fast:
                        nc.gpsimd.dma_start(out=den0[0:1, 0:2 * QW],
                                            in_=den64[64:65, :])

                def s_recip(st):
                    nc.vector.reciprocal_approx_fast(
                        out=rsb, in_=den0[0:1, 0:2 * QW])

                def s_cast(st):
                    # downcast the recip row (split over ScalarE/DVE) so the
                    # PE outer-products stream at bf16 rate (fp32 rhs is 2x
                    # slower); ~0.4% scale error, inside tolerance
                    nc.scalar.copy(out=rsbh[0:1, 0:QW], in_=rsb[0:1, 0:QW])
                    nc.vector.tensor_copy(out=rsbh[0:1, QW:2 * QW],
                                          in_=rsb[0:1, QW:2 * QW])

                def s_bcast(st):
                    # both recip rows broadcast by PE outer-products into the
                    # unused ot_a partitions 64..127 (A) and vacated den row
                    # region of ot_b (B: partitions 65..127 are free there)
                    nc.tensor.matmul(ot_a[64:128, :], lhsT=ones_rh,
                                     rhs=rsbh[0:1, 0:QW],
                                     start=True, stop=True)
                    nc.tensor.matmul(ot_b[64:128, :], lhsT=ones_rh,
                                     rhs=rsbh[0:1, QW:2 * QW],
                                     start=True, stop=True)

                def s_evac_a(st):
                    # ScalarE stages the broadcast rows to SBUF (lane-locked
                    # partition-preserving copies) - DVE ops may read only one
                    # PSUM operand
                    nc.scalar.copy(out=rbc[64:128, :], in_=ot_a[64:128, :])

                def s_evac_b(st):
                    nc.scalar.copy(out=rbc[0:64, :], in_=ot_b[64:128, :])

                def s_mul_a(st):
                    # 32-wide DVE ops: quadrant-aligned src windows may route
                    # to any output quadrant
                    nc.vector.tensor_mul(out=dst[0:32, :],
                                         in0=ot_a[0:32, :],
                                         in1=rbc[64:96, :])
                    nc.vector.tensor_mul(out=dst[32:64, :],
                                         in0=ot_a[32:64, :],
                                         in1=rbc[96:128, :])

                def s_mul_b(st):
                    # 64-wide quadrant-pair-aligned op crossing into 64..127
                    nc.vector.tensor_mul(out=dst[64:128, :],
                                         in0=ot_b[0:64, :],
                                         in1=rbc[0:64, :])

                steps = [s_den_a, s_den_b, s_dma, s_recip, s_cast,
                         s_bcast, s_evac_a, s_evac_b, s_mul_a, s_mul_b]
                if not fast:
                    steps = [s_dummy] + steps
                return steps, den0, rsb

            # kb slots (in the NEXT pair) at which each deferred step emits:
            # late enough that by the time the DVE reaches the recip/mul in
            # its queue, the serial cross-engine chain has already completed
            NORM_SLOTS = (5, 6, 7, 8, 9, 10, 11, 12, 13, 14, 15)
            norm_steps = []
            for qs in range(QS):
                for p in range(PAIRS):
                    kT = qk_t[("k", p)]
                    qT = qk_t[("q", p)]
                    last_pair = (qs == QS - 1 and p == PAIRS - 1)
                    ot_a = psO.tile([128, QW], F32, name="ot_a", tag="ot")
                    ot_b = psO.tile([128, QW], F32, name="ot_b", tag="ot")
                    pend = {}
                    for kb in range(TOK + PEND):
                        if kb < TOK:
                            st = psS.tile([128, 2 * QW], F32,
                                          name="st", tag="st")
                            st_a = st[:, 0:QW]
                            st_b = st[:, QW:2 * QW]
                            # scores for both heads into one PSUM tile; K=64
                            # row tiling runs the two matmuls concurrently
                            nc.tensor.matmul(
                                st_a,
                                lhsT=kT[0:64, 128 * kb:128 * (kb + 1)],
                                rhs=qT[0:64, QW * qs:QW * (qs + 1)],
                                start=True, stop=True,
                            )
                            nc.tensor.matmul(
                                st_b,
                                lhsT=kT[64:128, 128 * kb:128 * (kb + 1)],
                                rhs=qT[64:128, QW * qs:QW * (qs + 1)],
                                start=True, stop=True,
                            )
                            # one full-width exp per k-block, alternating
                            # engines (exact table exp on ScalarE /
                            # Schraudolph on the DVE): halves per-instruction
                            # overhead vs per-head splits. Tiles are written
                            # in their native dtype, READ through bitcast
                            if kb % 2 == 0:
                                pt_e = ptpool.tile([128, 2 * QW], BF16,
                                                   name="pt_e", tag="pte")
                                nc.scalar.activation(
                                    out=pt_e, in_=st, func=EXP, scale=0.125)
                                pend[kb] = (pt_e[:, 0:QW], pt_e[:, QW:2 * QW])
                            else:
                                pt_v = ptpool.tile([128, 2 * QW], I16,
                                                   name="pt_v", tag="ptv")
                                nc.vector.tensor_scalar(
                                    out=pt_v, in0=st,
                                    scalar1=EXPA, scalar2=EXPB,
                                    op0=MUL, op1=ADD)
                                pvb = pt_v.bitcast(BF16)
                                pend[kb] = (pvb[:, 0:QW], pvb[:, QW:2 * QW])
                            if norm_steps and kb in NORM_SLOTS:
                                norm_steps[NORM_SLOTS.index(kb)](st)
                                if kb == NORM_SLOTS[-1]:
                                    norm_steps = []
                        if kb >= PEND:
                            kv = kb - PEND
                            pa, pb = pend.pop(kv)
                            # O accumulation (64 V cols + ones col per head)
                            nc.tensor.matmul(
                                ot_a[0:65, :],
                                lhsT=v_t[kv][:, 2 * p, :],
                                rhs=pa,
                                start=(kv == 0), stop=(kv == TOK - 1),
                            )
                            nc.tensor.matmul(
                                ot_b[0:65, :],
                                lhsT=v_t[kv][:, 2 * p + 1, :],
                                rhs=pb,
                                start=(kv == 0), stop=(kv == TOK - 1),
                            )
                    # ---- normalization ---------------------------------
                    if last_pair:
                        # emitted immediately, per-head pipelined chains so
                        # the final projection's second accumulation half can
                        # start as soon as possible (this latency is the
                        # kernel's exposed tail)
                        den0 = rpool.tile([1, 2 * QW], F32,
                                          name="den0", tag="den0")
                        rsb = rpool.tile([1, 2 * QW], F32,
                                         name="rsb", tag="rsb")
                        rbc = rpool.tile([128, QW], F32, name="rbc", tag="rbc")
                        dst = ot_sb[p][qs]
                        # head A chain
                        nc.vector.tensor_copy(out=den0[0:1, 0:QW],
                                              in_=ot_a[64:65, :])
                        nc.vector.reciprocal_approx_fast(
                            out=rsb[0:1, 0:QW], in_=den0[0:1, 0:QW])
                        nc.tensor.matmul(ot_a[64:128, :], lhsT=ones_r,
                                         rhs=rsb[0:1, 0:QW],
                                         start=True, stop=True)
                        nc.scalar.copy(out=rbc[64:128, :], in_=ot_a[64:128, :])
                        nc.vector.tensor_mul(out=dst[0:32, :],
                                             in0=ot_a[0:32, :],
                                             in1=rbc[64:96, :])
                        nc.vector.tensor_mul(out=dst[32:64, :],
                                             in0=ot_a[32:64, :],
                                             in1=rbc[96:128, :])
                        # head B chain
                        nc.vector.tensor_copy(out=den0[0:1, QW:2 * QW],
                                              in_=ot_b[64:65, :])
                        nc.vector.reciprocal_approx_fast(
                            out=rsb[0:1, QW:2 * QW], in_=den0[0:1, QW:2 * QW])
                        if KEEPWARM:
                            kw = psS.tile([128, 2 * QW], F32,
                                          name="kw", tag="st")
                            nc.tensor.matmul(kw[0:64, 0:64],
                                             lhsT=den0[0:1, 0:64],
                                             rhs=den0[0:1, QW:QW + 64],
                                             start=True, stop=True)
                        nc.tensor.matmul(ot_b[64:128, :], lhsT=ones_r,
                                         rhs=rsb[0:1, QW:2 * QW],
                                         start=True, stop=True)
                        nc.scalar.copy(out=rbc[0:64, :], in_=ot_b[64:128, :])
                        nc.vector.tensor_mul(out=dst[64:128, :],
                                             in0=ot_b[0:64, :],
                                             in1=rbc[0:64, :])
                    else:
                        assert not norm_steps
                        norm_steps, _, _ = emit_norm(p, qs, ot_a, ot_b,
                                                     fast=False)

                # next query chunk's output projection (inputs long since
                # ready -> no PE stall)
                if qs > 0:
                    proj_chunk(qs - 1)
            proj_chunk(QS - 1, last=True)

    nc.compile()
    return nc


def _get_nc():
    if "nc" not in _cache:
        _cache["nc"] = _build_nc()
    return _cache["nc"]


def kernel(x, w_qkv, w_proj, b_proj):
    import ml_dtypes
    from concourse.bass_utils import run_bass_kernel_spmd

    x = np.asarray(x, dtype=np.float32)
    w_qkv = np.asarray(w_qkv, dtype=np.float32)
    w_proj = np.asarray(w_proj, dtype=np.float32)
    b_proj = np.asarray(b_proj, dtype=np.float32)

    nc = _get_nc()
    in_maps = []
    for core in range(NCORES):
        b, g = divmod(core, 4)
        rows = np.concatenate([
            np.arange(C * j + HL * D * g, C * j + HL * D * (g + 1))
            for j in range(3)
        ])
        wT = np.ascontiguousarray(w_qkv[rows].T)          # [1024, 768]
        xT = np.ascontiguousarray(x[b].T)                 # [1024, 2048]
        wp = np.ascontiguousarray(
            w_proj[:, HL * D * g:HL * D * (g + 1)].T)     # [256, 1024]
        x3 = xT.reshape(CT, 128, L).transpose(1, 0, 2)
        m = {
            f"xTb{i}": np.ascontiguousarray(
                x3[:, :, QW * i:QW * (i + 1)]).astype(ml_dtypes.bfloat16)
            for i in range(1, 4)
        }
        m["xTb0a"] = np.ascontiguousarray(
            x3[:, 0:4, 0:QW]).astype(ml_dtypes.bfloat16)
        m["xTb0b"] = np.ascontiguousarray(
            x3[:, 4:8, 0:QW]).astype(ml_dtypes.bfloat16)
        m.update({
            "wqkT": np.ascontiguousarray(
                wT[:, 0:2 * HL * D].reshape(CT, 128, 2 * HL * D)
                .transpose(1, 0, 2)
            ).astype(ml_dtypes.bfloat16),
            "wvT": np.ascontiguousarray(
                wT[:, 2 * HL * D:3 * HL * D].reshape(CT, 128, HL * D)
                .transpose(1, 0, 2)
            ).astype(ml_dtypes.bfloat16),
            "wpT": np.ascontiguousarray(
                wp.reshape(PAIRS, 128, C).transpose(1, 0, 2)
            ).astype(ml_dtypes.bfloat16),
        })
        in_maps.append(m)

    res = run_bass_kernel_spmd(
        nc, in_maps, list(range(NCORES)),
        trace=bool(os.environ.get("KERNEL_TRACE")),
    )
    _cache["last_results"] = res

    out = np.empty((B, L, C), dtype=np.float32)
    for b in range(B):
        acc = res.results[4 * b]["out"].astype(np.float32)
        for g in range(1, 4):
            acc = acc + res.results[4 * b + g]["out"]
        out[b] = acc + b_proj[None, :]
    return out


# revision 56
# speedup vs baseline: 1.0045x; 1.0045x over previous
"""Multi-head attention (B=2, L=2048, dim=1024, 16 heads) on 8 Trainium2 cores.

Sharding: 8 cores = 2 (batch) x 4 (head groups of 4 heads). Each core runs an
identical Bass program on its own slice (SPMD, no collectives); the host sums
the 4 per-head-group partial projection outputs per batch and adds the bias.

Per-core dataflow (bf16 matmul operands, fp32 PSUM accumulation):
  Inputs land via 7 large contiguous DMA descriptors, ALL issued on the sync
  queue in need-order: SDMA engines round-robin across queues-with-work, so
  a single FIFO queue gives the first-needed bytes the full aggregate
  bandwidth instead of an equal share.
  V token-major [128 tok, 4 heads, 64+1] (ones column fused for the softmax
    denominator), qT/kT feature-major [128 (2 heads x 64d), 2048]
  Per k-block: score tile ST[k, q] for both heads (the two K=64 matmuls run
    concurrently via PE row tiling); one full-width exp per k-block,
    alternating engines (exact table exp on ScalarE / Schraudolph bit-trick
    on the DVE: bits = S*A + B as int16, bitcast to bf16, ~3% relative,
    cancels in the softmax ratio).
  OT_h[d, q] += V_h.T @ PT_h   (M=65: row 64 accumulates the denominator)
  normalization (the critical redesign vs the 261us version): pair N's chain
    is emitted step-by-step inside pair N+1's k-loop (slots kb=6..15) with a
    dummy-element gate on pair N+1's kb-6 score tile, so the Tile
    scheduler's ready-heap cannot place these long-latency ops ahead of
    pair N+1's exps in the DVE queue (head-of-line blocking there stalled
    the PE ~4us at every pair boundary). The GpSimd partition_broadcast
    (constant ~4.5us queue wait) is replaced by PE outer-products
    (ones[1,64].T @ recip row) into the unused partitions 64..127 of the OT
    PSUM banks; ScalarE stages them to SBUF (DVE ops may read only one PSUM
    operand) and the muls run on the DVE (32-wide ops for head A, 64-wide
    quadrant-pair-aligned op for head B crossing into partitions 64..127).
    Denominator rows: ScalarE copies (partition 64) + one SBUF-SBUF DMA to
    partition 0 + reciprocal_approx_fast; the last pair instead uses DVE
    cross-quadrant copies and per-head pipelined chains to shorten the
    exposed kernel tail.
  out[tok, c] = OT_norm.T @ wpT -> PSUM, staged to fp16 SBUF, DMA'd to DRAM
    on two alternating issue queues.
"""

import os
import math
import numpy as np

B, L, C = 2, 2048, 1024
H, D = 16, 64
HL = 4            # heads per core (local)
PAIRS = 2         # head pairs per core
CT = C // 128     # 8 contraction tiles for the projections
TOK = L // 128    # 16 key-token tiles
QW = 512          # query tile width
QS = L // QW      # 4 query tiles
NCORES = 8
PEND = 3          # k-blocks of slack between a PV matmul and its exp

# Schraudolph exp constants (bf16 bit pattern): bits = S_raw * EXPA + EXPB
EXPA = 0.125 * math.log2(math.e) * 128.0
EXPB = 128.0 * (127.0 - 0.043035)
KEEPWARM = False
LASTPAIR_DVE_DEN = True   # DVE cross-quadrant copy of den rows for last pair

_cache = {}


def _build_nc():
    import concourse.bass as bass
    import concourse.mybir as mybir
    import concourse.tile as tile
    from concourse import bacc

    F32 = mybir.dt.float32
    BF16 = mybir.dt.bfloat16
    F16 = mybir.dt.float16
    I16 = mybir.dt.int16
    EXP = mybir.ActivationFunctionType.Exp
    MUL = mybir.AluOpType.mult
    ADD = mybir.AluOpType.add

    nc = bacc.Bacc("TRN2", target_bir_lowering=False, debug=False,
                   num_devices=NCORES)

    # channel-tile-major 3D layouts prepared on the host so each load is one
    # big strided descriptor
    xTb = [nc.declare_dram_parameter(f"xTb{i}", [128, CT, QW], BF16,
                                       isOutput=False) for i in range(1, 4)]
    xTb0 = [nc.declare_dram_parameter(f"xTb0{h}", [128, CT // 2, QW], BF16,
                                      isOutput=False) for h in ("a", "b")]
    wqkT = nc.declare_dram_parameter("wqkT", [128, CT, 2 * HL * D], BF16,
                                     isOutput=False)
    wvT = nc.declare_dram_parameter("wvT", [128, CT, HL * D], BF16,
                                    isOutput=False)
    wpT = nc.declare_dram_parameter("wpT", [128, PAIRS, C], BF16,
                                    isOutput=False)
    out = nc.declare_dram_parameter("out", [L, C], F16, isOutput=True)

    with tile.TileContext(nc) as tc:
        from contextlib import ExitStack
        with ExitStack() as ctx:
            qkpool = ctx.enter_context(tc.tile_pool(name="qk", bufs=1))
            vpool = ctx.enter_context(tc.tile_pool(name="v", bufs=1))
            wppool = ctx.enter_context(tc.tile_pool(name="wp", bufs=1))
            # per-head score tiles: 2 bufs per head tag = 4 PSUM banks
            psS = ctx.enter_context(tc.tile_pool(name="psS", bufs=2, space="PSUM"))
            phase1 = ExitStack()
            xpool = phase1.enter_context(tc.tile_pool(name="x", bufs=1))
            wpool = phase1.enter_context(tc.tile_pool(name="w", bufs=1))
            psA = phase1.enter_context(tc.tile_pool(name="psA", bufs=2, space="PSUM"))

            # ---- input loads: 7 large descriptors spread over 3 queues.
            # x is split into 4 column blocks (one tile per block so the
            # tile-granular DMA dependency gates only its own consumers).
            x_b = [None] + [xpool.tile([128, CT, QW], BF16,
                                       name=f"xb{i}", tag=f"xb{i}")
                            for i in range(1, 4)]
            x_b0 = [xpool.tile([128, CT // 2, QW], BF16,
                               name=f"xb0{h}", tag=f"xb0{h}")
                    for h in range(2)]

            def xs(blk, c):
                # channel-tile c of x column block blk (block 0 is split in
                # two c-halves so the first compute gates on half the bytes)
                if blk == 0:
                    return x_b0[c // 4][:, c % 4, :]
                return x_b[blk][:, c, :]
            wv_sb = wpool.tile([128, CT, HL * D], BF16, name="wv", tag="wv")
            wqk_sb = wpool.tile([128, CT, 2 * HL * D], BF16, name="wqk", tag="wqk")
            wp_all = wppool.tile([128, PAIRS, C], BF16, name="wp", tag="wp")

            # ALL input loads on one queue, FIFO in need-order: with a single
            # queue holding DMA work, every SDMA engine drains it exclusively,
            # so the first-needed bytes get the full aggregate bandwidth
            # (spreading across issue queues gives each an equal share and
            # starves the critical first loads behind non-critical ones)
            nc.sync.dma_start(out=wv_sb, in_=wvT[:, :, :])
            nc.sync.dma_start(out=x_b0[0], in_=xTb0[0][:, :, :])
            nc.sync.dma_start(out=x_b0[1], in_=xTb0[1][:, :, :])
            nc.sync.dma_start(out=wqk_sb, in_=wqkT[:, :, :])
            nc.sync.dma_start(out=x_b[1], in_=xTb[0][:, :, :])
            nc.sync.dma_start(out=x_b[2], in_=xTb[1][:, :, :])
            nc.sync.dma_start(out=x_b[3], in_=xTb[2][:, :, :])
            nc.sync.dma_start(out=wp_all, in_=wpT[:, :, :])

            # ---- V token-major: v[t] = [128 tok, HL, D+1] (ones col fused) --
            ones_s = vpool.tile([128, HL, 1], F32, name="ones_s", tag="ones_s")
            nc.vector.memset(ones_s, 1.0)
            # [1, 64] ones row: stationary operand of the PE outer-product
            # that broadcasts the softmax reciprocals across partitions
            ones_r = vpool.tile([1, 64], F32, name="ones_r", tag="ones_r")
            nc.vector.memset(ones_r, 1.0)
            ones_rh = vpool.tile([1, 64], BF16, name="ones_rh", tag="ones_rh")
            nc.vector.memset(ones_rh, 1.0)
            v_t = [vpool.tile([128, HL, D + 1], BF16, name=f"v{t}", tag=f"v{t}")
                   for t in range(TOK)]

            def vchunk(t):
                ps = psA.tile([128, HL * D], F32, name="psv", tag="ps")
                tc4 = t % 4
                for c in range(CT):
                    nc.tensor.matmul(
                        ps,
                        lhsT=xs(t // 4, c)[:, 128 * tc4:128 * (tc4 + 1)],
                        rhs=wv_sb[:, c, :],
                        start=(c == 0), stop=(c == CT - 1),
                    )
                vt = v_t[t]
                nc.vector.tensor_copy(out=vt[:, :, D:D + 1], in_=ones_s)
                nc.vector.tensor_copy(
                    out=vt[:, :, 0:D],
                    in_=ps.rearrange("p (h d) -> p h d", h=HL),
                )

            # ---- Q/K feature-major per pair: [128 (2h x 64d), L] ------------
            qk_t = {}
            for p in range(PAIRS):
                for nm in ("q", "k"):
                    qk_t[(nm, p)] = qkpool.tile(
                        [128, L], BF16, name=f"{nm}{p}", tag=f"{nm}{p}")

            qk_evac = [0]

            def qkchunk(nm, p, ns):
                j = 0 if nm == "q" else 1
                ps = psA.tile([128, QW], F32, name="psqk", tag="ps")
                for c in range(CT):
                    nc.tensor.matmul(
                        ps,
                        lhsT=wqk_sb[:, c, j * HL * D + 128 * p:
                                    j * HL * D + 128 * (p + 1)],
                        rhs=xs(ns, c),
                        start=(c == 0), stop=(c == CT - 1),
                    )
                dst = qk_t[(nm, p)][:, QW * ns:QW * (ns + 1)]
                # alternate PSUM evacuation between ScalarE and the DVE
                if qk_evac[0] % 2 == 0:
                    nc.scalar.copy(out=dst, in_=ps)
                else:
                    nc.vector.tensor_copy(out=dst, in_=ps)
                qk_evac[0] += 1

            # emission grouped by which x column block each matmul needs
            for t in range(4):
                vchunk(t)
            qkchunk("k", 0, 0), qkchunk("k", 1, 0)
            for t in range(4, 8):
                vchunk(t)
            qkchunk("k", 0, 1), qkchunk("k", 1, 1)
            qkchunk("q", 0, 0), qkchunk("q", 1, 0)
            for t in range(8, 12):
                vchunk(t)
            qkchunk("k", 0, 2), qkchunk("k", 1, 2)
            qkchunk("q", 0, 1), qkchunk("q", 1, 1)
            for t in range(12, 16):
                vchunk(t)
            qkchunk("k", 0, 3), qkchunk("k", 1, 3)
            for ns in (2, 3):
                qkchunk("q", 0, ns), qkchunk("q", 1, ns)

            phase1.close()
            # ---- phase 2 pools (reuse the x/w SBUF + psA PSUM space) --------
            psO = ctx.enter_context(tc.tile_pool(name="psO", bufs=4, space="PSUM"))
            otpool = ctx.enter_context(tc.tile_pool(name="ot", bufs=1))
            ptpool = ctx.enter_context(tc.tile_pool(name="pt", bufs=PEND))
            rpool = ctx.enter_context(tc.tile_pool(name="r", bufs=2))
            obpool = ctx.enter_context(tc.tile_pool(name="ob", bufs=4))

            out_qs = [nc.sync, nc.gpsimd]
            _out_rr = [0]

            def proj_chunk(qs, last=False):
                # PSUM -> fp16 SBUF staging split across ScalarE/DVE, then out.
                # Output DMAs round-robin over four issue queues so the final
                # chunk's descriptor-issue time does not serialize the drain.
                for t in range(QW // 128 * qs, QW // 128 * (qs + 1)):
                    ob = obpool.tile([128, C], F16, name="ob", tag="ob")
                    for nh in range(C // QW):
                        ps = psO.tile([128, QW], F32, name="psp", tag="ot")
                        for p2 in range(PAIRS):
                            nc.tensor.matmul(
                                ps,
                                lhsT=ot_sb[p2][qs][:, 128 * (t % (QW // 128)):
                                                   128 * (t % (QW // 128) + 1)],
                                rhs=wp_all[:, p2, QW * nh:QW * (nh + 1)],
                                start=(p2 == 0), stop=(p2 == PAIRS - 1),
                            )
                        dst = ob[:, QW * nh:QW * (nh + 1)]
                        if last:
                            # engine-parallel half-copies: the final chunk's
                            # staging sits on the strictly serial endgame
                            # path and both engines are otherwise idling
                            h = QW // 2
                            nc.scalar.copy(out=dst[:, 0:h], in_=ps[:, 0:h])
                            nc.vector.tensor_copy(out=dst[:, h:QW],
                                                  in_=ps[:, h:QW])
                        elif nh == 0:
                            nc.scalar.copy(out=dst, in_=ps)
                        else:
                            nc.vector.tensor_copy(out=dst, in_=ps)
                        orow = out[128 * t:128 * (t + 1), :]
                        eng = out_qs[_out_rr[0] % len(out_qs)]
                        _out_rr[0] += 1
                        eng.dma_start(
                            out=orow[:, QW * nh:QW * (nh + 1)], in_=dst)

            # ---- attention --------------------------------------------------
            ot_sb = [[otpool.tile([128, QW], BF16, name=f"otp{p}q{q}",
                                  tag=f"otp{p}q{q}")
                      for q in range(QS)] for p in range(PAIRS)]

            def emit_norm(pp, pqs, ot_a, ot_b, fast):
                # Normalization WITHOUT GpSimd partition_broadcast (that op
                # showed a constant ~4.5us queue wait gating every pair
                # boundary). The reciprocal row is broadcast across
                # partitions by a PE outer-product (ones[1,64].T @ rsb) into
                # the unused partitions 64-127 / vacated 0-63 of the ot_a
                # PSUM bank; the muls run on the DVE reading PSUM directly.
                # fast=True (last pair): den rows hop to partition 0 via DVE
                # cross-quadrant copies; else ScalarE stages them at
                # partition 64 and one SBUF-SBUF DMA moves them to 0.
                den64 = rpool.tile([65, 2 * QW], F32, name="den64", tag="den64")
                den0 = rpool.tile([1, 2 * QW + 8], F32, name="den0", tag="den0")
                rsb = rpool.tile([1, 2 * QW], F32, name="rsb", tag="rsb")
                rsbh = rpool.tile([1, 2 * QW], BF16, name="rsbh", tag="rsbh")
                rbc = rpool.tile([128, QW], F32, name="rbc", tag="rbc")
                dst = ot_sb[pp][pqs]

                def s_dummy(st):
                    # real dependency on the NEXT pair's k-block: keeps the
                    # chain's DVE ops out of the queue until mid-pair
                    nc.vector.tensor_copy(
                        out=den0[0:1, 2 * QW:2 * QW + 1], in_=st[0:1, 0:1])

                def s_den_a(st):
                    if fast:
                        nc.vector.tensor_copy(out=den0[0:1, 0:QW],
                                              in_=ot_a[64:65, :])
                    else:
                        nc.scalar.copy(out=den64[64:65, 0:QW],
                                       in_=ot_a[64:65, :])

                def s_den_b(st):
                    if fast:
                        nc.vector.tensor_copy(out=den0[0:1, QW:2 * QW],
                                              in_=ot_b[64:65, :])
                    else:
                        nc.scalar.copy(out=den64[64:65, QW:2 * QW],
                                       in_=ot_b[64:65, :])

                def s_dma(st):
                    if not fast:
                        nc.gpsimd.dma_start(out=den0[0:1, 0:2 * QW],
                                            in_=den64[64:65, :])

                def s_recip(st):
                    nc.vector.reciprocal_approx_fast(
                        out=rsb, in_=den0[0:1, 0:2 * QW])

                def s_cast(st):
                    # downcast the recip row (split over ScalarE/DVE) so the
                    # PE outer-products stream at bf16 rate (fp32 rhs is 2x
                    # slower); ~0.4% scale error, inside tolerance
                    nc.scalar.copy(out=rsbh[0:1, 0:QW], in_=rsb[0:1, 0:QW])
                    nc.vector.tensor_copy(out=rsbh[0:1, QW:2 * QW],
                                          in_=rsb[0:1, QW:2 * QW])

                def s_bcast(st):
                    # both recip rows broadcast by PE outer-products into the
                    # unused ot_a partitions 64..127 (A) and vacated den row
                    # region of ot_b (B: partitions 65..127 are free there)
                    nc.tensor.matmul(ot_a[64:128, :], lhsT=ones_rh,
                                     rhs=rsbh[0:1, 0:QW],
                                     start=True, stop=True)
                    nc.tensor.matmul(ot_b[64:128, :], lhsT=ones_rh,
                                     rhs=rsbh[0:1, QW:2 * QW],
                                     start=True, stop=True)

                def s_evac_a(st):
                    # ScalarE stages the broadcast rows to SBUF (lane-locked
                    # partition-preserving copies) - DVE ops may read only one
                    # PSUM operand
                    nc.scalar.copy(out=rbc[64:128, :], in_=ot_a[64:128, :])

                def s_evac_b(st):
                    nc.scalar.copy(out=rbc[0:64, :], in_=ot_b[64:128, :])

                def s_mul_a(st):
                    # 32-wide DVE ops: quadrant-aligned src windows may route
                    # to any output quadrant
                    nc.vector.tensor_mul(out=dst[0:32, :],
                                         in0=ot_a[0:32, :],
                                         in1=rbc[64:96, :])
                    nc.vector.tensor_mul(out=dst[32:64, :],
                                         in0=ot_a[32:64, :],
                                         in1=rbc[96:128, :])

                def s_mul_b(st):
                    # 64-wide quadrant-pair-aligned op crossing into 64..127
                    nc.vector.tensor_mul(out=dst[64:128, :],
                                         in0=ot_b[0:64, :],
                                         in1=rbc[0:64, :])

                steps = [s_den_a, s_den_b, s_dma, s_recip, s_cast,
                         s_bcast, s_evac_a, s_evac_b, s_mul_a, s_mul_b]
                if not fast:
                    steps = [s_dummy] + steps
                return steps, den0, rsb

            # kb slots (in the NEXT pair) at which each deferred step emits:
            # late enough that by the time the DVE reaches the recip/mul in
            # its queue, the serial cross-engine chain has already completed
            NORM_SLOTS = (5, 6, 7, 8, 9, 10, 11, 12, 13, 14, 15)
            norm_steps = []
            for qs in range(QS):
                for p in range(PAIRS):
                    kT = qk_t[("k", p)]
                    qT = qk_t[("q", p)]
                    last_pair = (qs == QS - 1 and p == PAIRS - 1)
                    ot_a = psO.tile([128, QW], F32, name="ot_a", tag="ot")
                    ot_b = psO.tile([128, QW], F32, name="ot_b", tag="ot")
                    pend = {}
                    for kb in range(TOK + PEND):
                        if kb < TOK:
                            st = psS.tile([128, 2 * QW], F32,
                                          name="st", tag="st")
                            st_a = st[:, 0:QW]
                            st_b = st[:, QW:2 * QW]
                            # scores for both heads into one PSUM tile; K=64
                            # row tiling runs the two matmuls concurrently
                            nc.tensor.matmul(
                                st_a,
                                lhsT=kT[0:64, 128 * kb:128 * (kb + 1)],
                                rhs=qT[0:64, QW * qs:QW * (qs + 1)],
                                start=True, stop=True,
                            )
                            nc.tensor.matmul(
                                st_b,
                                lhsT=kT[64:128, 128 * kb:128 * (kb + 1)],
                                rhs=qT[64:128, QW * qs:QW * (qs + 1)],
                                start=True, stop=True,
                            )
                            # one full-width exp per k-block, alternating
                            # engines (exact table exp on ScalarE /
                            # Schraudolph on the DVE): halves per-instruction
                            # overhead vs per-head splits. Tiles are written
                            # in their native dtype, READ through bitcast
                            if kb % 2 == 0:
                                pt_e = ptpool.tile([128, 2 * QW], BF16,
                                                   name="pt_e", tag="pte")
                                nc.scalar.activation(
                                    out=pt_e, in_=st, func=EXP, scale=0.125)
                                pend[kb] = (pt_e[:, 0:QW], pt_e[:, QW:2 * QW])
                            else:
                                pt_v = ptpool.tile([128, 2 * QW], I16,
                                                   name="pt_v", tag="ptv")
                                nc.vector.tensor_scalar(
                                    out=pt_v, in0=st,
                                    scalar1=EXPA, scalar2=EXPB,
                                    op0=MUL, op1=ADD)
                                pvb = pt_v.bitcast(BF16)
                                pend[kb] = (pvb[:, 0:QW], pvb[:, QW:2 * QW])
                            if norm_steps and kb in NORM_SLOTS:
                                norm_steps[NORM_SLOTS.index(kb)](st)
                                if kb == NORM_SLOTS[-1]:
                                    norm_steps = []
                        if kb >= PEND:
                            kv = kb - PEND
                            pa, pb = pend.pop(kv)
                            # O accumulation (64 V cols + ones col per head)
                            nc.tensor.matmul(
                                ot_a[0:65, :],
                                lhsT=v_t[kv][:, 2 * p, :],
                                rhs=pa,
                                start=(kv == 0), stop=(kv == TOK - 1),
                            )
                            nc.tensor.matmul(
                                ot_b[0:65, :],
                                lhsT=v_t[kv][:, 2 * p + 1, :],
                                rhs=pb,
                                start=(kv == 0), stop=(kv == TOK - 1),
                            )
                    # ---- normalization ---------------------------------
                    if last_pair:
                        # emitted immediately, per-head pipelined chains so
                        # the final projection's second accumulation half can
                        # start as soon as possible (this latency is the
                        # kernel's exposed tail)
                        den0 = rpool.tile([1, 2 * QW], F32,
                                          name="den0", tag="den0")
                        rsb = rpool.tile([1, 2 * QW], F32,
                                         name="rsb", tag="rsb")
                        rbc = rpool.tile([128, QW], F32, name="rbc", tag="rbc")
                        dst = ot_sb[p][qs]
                        # head A chain
                        nc.vector.tensor_copy(out=den0[0:1, 0:QW],
                                              in_=ot_a[64:65, :])
                        nc.vector.reciprocal_approx_fast(
                            out=rsb[0:1, 0:QW], in_=den0[0:1, 0:QW])
                        nc.tensor.matmul(ot_a[64:128, :], lhsT=ones_r,
                                         rhs=rsb[0:1, 0:QW],
                                         start=True, stop=True)
                        nc.scalar.copy(out=rbc[64:128, :], in_=ot_a[64:128, :])
                        nc.vector.tensor_mul(out=dst[0:32, :],
                                             in0=ot_a[0:32, :],
                                             in1=rbc[64:96, :])
                        nc.vector.tensor_mul(out=dst[32:64, :],
                                             in0=ot_a[32:64, :],
                                             in1=rbc[96:128, :])
                        # head B chain
                        nc.vector.tensor_copy(out=den0[0:1, QW:2 * QW],
                                              in_=ot_b[64:65, :])
                        nc.vector.reciprocal_approx_fast(
                            out=rsb[0:1, QW:2 * QW], in_=den0[0:1, QW:2 * QW])
                        if KEEPWARM:
                            kw = psS.tile([128, 2 * QW], F32,
                                          name="kw", tag="st")
                            nc.tensor.matmul(kw[0:64, 0:64],
                                             lhsT=den0[0:1, 0:64],
                                             rhs=den0[0:1, QW:QW + 64],
                                             start=True, stop=True)
                        nc.tensor.matmul(ot_b[64:128, :], lhsT=ones_r,
                                         rhs=rsb[0:1, QW:2 * QW],
                                         start=True, stop=True)
                        nc.scalar.copy(out=rbc[0:64, :], in_=ot_b[64:128, :])
                        nc.vector.tensor_mul(out=dst[64:128, :],
                                             in0=ot_b[0:64, :],
                                             in1=rbc[0:64, :])
                    else:
                        assert not norm_steps
                        norm_steps, _, _ = emit_norm(p, qs, ot_a, ot_b,
                                                     fast=False)

                # next query chunk's output projection (inputs long since
                # ready -> no PE stall)
                if qs > 0:
                    proj_chunk(qs - 1)
            proj_chunk(QS - 1, last=True)

    nc.compile()
    return nc


def _get_nc():
    if "nc" not in _cache:
        _cache["nc"] = _build_nc()
    return _cache["nc"]


def kernel(x, w_qkv, w_proj, b_proj):
    import ml_dtypes
    from concourse.bass_utils import run_bass_kernel_spmd

    x = np.asarray(x, dtype=np.float32)
    w_qkv = np.asarray(w_qkv, dtype=np.float32)
    w_proj = np.asarray(w_proj, dtype=np.float32)
    b_proj = np.asarray(b_proj, dtype=np.float32)

    nc = _get_nc()
    in_maps = []
    for core in range(NCORES):
        b, g = divmod(core, 4)
        rows = np.concatenate([
            np.arange(C * j + HL * D * g, C * j + HL * D * (g + 1))
            for j in range(3)
        ])
        wT = np.ascontiguousarray(w_qkv[rows].T)          # [1024, 768]
        xT = np.ascontiguousarray(x[b].T)                 # [1024, 2048]
        wp = np.ascontiguousarray(
            w_proj[:, HL * D * g:HL * D * (g + 1)].T)     # [256, 1024]
        x3 = xT.reshape(CT, 128, L).transpose(1, 0, 2)
        m = {
            f"xTb{i}": np.ascontiguousarray(
                x3[:, :, QW * i:QW * (i + 1)]).astype(ml_dtypes.bfloat16)
            for i in range(1, 4)
        }
        m["xTb0a"] = np.ascontiguousarray(
            x3[:, 0:4, 0:QW]).astype(ml_dtypes.bfloat16)
        m["xTb0b"] = np.ascontiguousarray(
            x3[:, 4:8, 0:QW]).astype(ml_dtypes.bfloat16)
        m.update({
            "wqkT": np.ascontiguousarray(
                wT[:, 0:2 * HL * D].reshape(CT, 128, 2 * HL * D)
                .transpose(1, 0, 2)
            ).astype(ml_dtypes.bfloat16),
            "wvT": np.ascontiguousarray(
                wT[:, 2 * HL * D:3 * HL * D].reshape(CT, 128, HL * D)
                .transpose(1, 0, 2)
            ).astype(ml_dtypes.bfloat16),
            "wpT": np.ascontiguousarray(
                wp.reshape(PAIRS, 128, C).transpose(1, 0, 2)
            ).astype(ml_dtypes.bfloat16),
        })
        in_maps.append(m)

    res = run_bass_kernel_spmd(
        nc, in_maps, list(range(NCORES)),
        trace=bool(os.environ.get("KERNEL_TRACE")),
    )
    _cache["last_results"] = res

    out = np.empty((B, L, C), dtype=np.float32)
    for b in range(B):
        acc = res.results[4 * b]["out"].astype(np.float32)
        for g in range(1, 4):
            acc = acc + res.results[4 * b + g]["out"]
        out[b] = acc + b_proj[None, :]
    return out


# revision 57
# speedup vs baseline: 1.0147x; 1.0102x over previous
"""Multi-head attention (B=2, L=2048, dim=1024, 16 heads) on 8 Trainium2 cores.

Sharding: 8 cores = 2 (batch) x 4 (head groups of 4 heads). Each core runs an
identical Bass program on its own slice (SPMD, no collectives); the host sums
the 4 per-head-group partial projection outputs per batch and adds the bias.

Per-core dataflow (bf16 matmul operands, fp32 PSUM accumulation):
  Inputs land via 7 large contiguous DMA descriptors, ALL issued on the sync
  queue in need-order: SDMA engines round-robin across queues-with-work, so
  a single FIFO queue gives the first-needed bytes the full aggregate
  bandwidth instead of an equal share.
  V token-major [128 tok, 4 heads, 64+1] (ones column fused for the softmax
    denominator), qT/kT feature-major [128 (2 heads x 64d), 2048]
  Per k-block: score tile ST[k, q] for both heads (the two K=64 matmuls run
    concurrently via PE row tiling); one full-width exp per k-block,
    alternating engines (exact table exp on ScalarE / Schraudolph bit-trick
    on the DVE: bits = S*A + B as int16, bitcast to bf16, ~3% relative,
    cancels in the softmax ratio).
  OT_h[d, q] += V_h.T @ PT_h   (M=65: row 64 accumulates the denominator)
  normalization (the critical redesign vs the 261us version): pair N's chain
    is emitted step-by-step inside pair N+1's k-loop (slots kb=6..15) with a
    dummy-element gate on pair N+1's kb-6 score tile, so the Tile
    scheduler's ready-heap cannot place these long-latency ops ahead of
    pair N+1's exps in the DVE queue (head-of-line blocking there stalled
    the PE ~4us at every pair boundary). The GpSimd partition_broadcast
    (constant ~4.5us queue wait) is replaced by PE outer-products
    (ones[1,64].T @ recip row) into the unused partitions 64..127 of the OT
    PSUM banks; ScalarE stages them to SBUF (DVE ops may read only one PSUM
    operand) and the muls run on the DVE (32-wide ops for head A, 64-wide
    quadrant-pair-aligned op for head B crossing into partitions 64..127).
    Denominator rows: ScalarE copies (partition 64) + one SBUF-SBUF DMA to
    partition 0 + reciprocal_approx_fast; the last pair instead uses DVE
    cross-quadrant copies and per-head pipelined chains to shorten the
    exposed kernel tail.
  out[tok, c] = OT_norm.T @ wpT -> PSUM, staged to fp16 SBUF, DMA'd to DRAM
    on two alternating issue queues.
"""

import os
import math
import numpy as np

B, L, C = 2, 2048, 1024
H, D = 16, 64
HL = 4            # heads per core (local)
PAIRS = 2         # head pairs per core
CT = C // 128     # 8 contraction tiles for the projections
TOK = L // 128    # 16 key-token tiles
QW = 512          # query tile width
QS = L // QW      # 4 query tiles
NCORES = 8
PEND = 3          # k-blocks of slack between a PV matmul and its exp

# Schraudolph exp constants (bf16 bit pattern): bits = S_raw * EXPA + EXPB
EXPA = 0.125 * math.log2(math.e) * 128.0
EXPB = 128.0 * (127.0 - 0.043035)
KEEPWARM = False
LASTPAIR_DVE_DEN = True   # DVE cross-quadrant copy of den rows for last pair

_cache = {}


def _build_nc():
    import concourse.bass as bass
    import concourse.mybir as mybir
    import concourse.tile as tile
    from concourse import bacc

    F32 = mybir.dt.float32
    BF16 = mybir.dt.bfloat16
    F16 = mybir.dt.float16
    I16 = mybir.dt.int16
    EXP = mybir.ActivationFunctionType.Exp
    MUL = mybir.AluOpType.mult
    ADD = mybir.AluOpType.add

    nc = bacc.Bacc("TRN2", target_bir_lowering=False, debug=False,
                   num_devices=NCORES)

    # channel-tile-major 3D layouts prepared on the host so each load is one
    # big strided descriptor
    xTb = [nc.declare_dram_parameter(f"xTb{i}", [128, CT, QW], BF16,
                                       isOutput=False) for i in range(1, 4)]
    xTb0 = [nc.declare_dram_parameter(f"xTb0{h}", [128, CT // 2, QW], BF16,
                                      isOutput=False) for h in ("a", "b")]
    wqkT = nc.declare_dram_parameter("wqkT", [128, CT, 2 * HL * D], BF16,
                                     isOutput=False)
    wvT = nc.declare_dram_parameter("wvT", [128, CT, HL * D], BF16,
                                    isOutput=False)
    wpT = nc.declare_dram_parameter("wpT", [128, PAIRS, C], BF16,
                                    isOutput=False)
    out = nc.declare_dram_parameter("out", [L, C], F16, isOutput=True)

    with tile.TileContext(nc) as tc:
        from contextlib import ExitStack
        with ExitStack() as ctx:
            qkpool = ctx.enter_context(tc.tile_pool(name="qk", bufs=1))
            vpool = ctx.enter_context(tc.tile_pool(name="v", bufs=1))
            wppool = ctx.enter_context(tc.tile_pool(name="wp", bufs=1))
            # per-head score tiles: 2 bufs per head tag = 4 PSUM banks
            psS = ctx.enter_context(tc.tile_pool(name="psS", bufs=2, space="PSUM"))
            phase1 = ExitStack()
            xpool = phase1.enter_context(tc.tile_pool(name="x", bufs=1))
            wpool = phase1.enter_context(tc.tile_pool(name="w", bufs=1))
            psA = phase1.enter_context(tc.tile_pool(name="psA", bufs=2, space="PSUM"))

            # ---- input loads: 7 large descriptors spread over 3 queues.
            # x is split into 4 column blocks (one tile per block so the
            # tile-granular DMA dependency gates only its own consumers).
            x_b = [None] + [xpool.tile([128, CT, QW], BF16,
                                       name=f"xb{i}", tag=f"xb{i}")
                            for i in range(1, 4)]
            x_b0 = [xpool.tile([128, CT // 2, QW], BF16,
                               name=f"xb0{h}", tag=f"xb0{h}")
                    for h in range(2)]

            def xs(blk, c):
                # channel-tile c of x column block blk (block 0 is split in
                # two c-halves so the first compute gates on half the bytes)
                if blk == 0:
                    return x_b0[c // 4][:, c % 4, :]
                return x_b[blk][:, c, :]
            wv_sb = wpool.tile([128, CT, HL * D], BF16, name="wv", tag="wv")
            wqk_sb = wpool.tile([128, CT, 2 * HL * D], BF16, name="wqk", tag="wqk")
            wp_all = wppool.tile([128, PAIRS, C], BF16, name="wp", tag="wp")

            # ALL input loads on one queue, FIFO in need-order: with a single
            # queue holding DMA work, every SDMA engine drains it exclusively,
            # so the first-needed bytes get the full aggregate bandwidth
            # (spreading across issue queues gives each an equal share and
            # starves the critical first loads behind non-critical ones)
            nc.sync.dma_start(out=wv_sb, in_=wvT[:, :, :])
            nc.sync.dma_start(out=x_b0[0], in_=xTb0[0][:, :, :])
            nc.sync.dma_start(out=x_b0[1], in_=xTb0[1][:, :, :])
            nc.sync.dma_start(out=wqk_sb, in_=wqkT[:, :, :])
            nc.sync.dma_start(out=x_b[1], in_=xTb[0][:, :, :])
            nc.sync.dma_start(out=x_b[2], in_=xTb[1][:, :, :])
            nc.sync.dma_start(out=x_b[3], in_=xTb[2][:, :, :])
            nc.sync.dma_start(out=wp_all, in_=wpT[:, :, :])

            # ---- V token-major: v[t] = [128 tok, HL, D+1] (ones col fused) --
            ones_s = vpool.tile([128, HL, 1], F32, name="ones_s", tag="ones_s")
            nc.vector.memset(ones_s, 1.0)
            # [1, 64] ones row: stationary operand of the PE outer-product
            # that broadcasts the softmax reciprocals across partitions
            ones_r = vpool.tile([1, 64], F32, name="ones_r", tag="ones_r")
            nc.vector.memset(ones_r, 1.0)
            ones_rh = vpool.tile([1, 64], BF16, name="ones_rh", tag="ones_rh")
            nc.vector.memset(ones_rh, 1.0)
            v_t = [vpool.tile([128, HL, D + 1], BF16, name=f"v{t}", tag=f"v{t}")
                   for t in range(TOK)]

            def vchunk(t):
                ps = psA.tile([128, HL * D], F32, name="psv", tag="ps")
                tc4 = t % 4
                for c in range(CT):
                    nc.tensor.matmul(
                        ps,
                        lhsT=xs(t // 4, c)[:, 128 * tc4:128 * (tc4 + 1)],
                        rhs=wv_sb[:, c, :],
                        start=(c == 0), stop=(c == CT - 1),
                    )
                vt = v_t[t]
                nc.vector.tensor_copy(out=vt[:, :, D:D + 1], in_=ones_s)
                nc.vector.tensor_copy(
                    out=vt[:, :, 0:D],
                    in_=ps.rearrange("p (h d) -> p h d", h=HL),
                )

            # ---- Q/K feature-major per pair: [128 (2h x 64d), L] ------------
            qk_t = {}
            for p in range(PAIRS):
                for nm in ("q", "k"):
                    qk_t[(nm, p)] = qkpool.tile(
                        [128, L], BF16, name=f"{nm}{p}", tag=f"{nm}{p}")

            qk_evac = [0]

            def qkchunk(nm, p, ns):
                j = 0 if nm == "q" else 1
                ps = psA.tile([128, QW], F32, name="psqk", tag="ps")
                for c in range(CT):
                    nc.tensor.matmul(
                        ps,
                        lhsT=wqk_sb[:, c, j * HL * D + 128 * p:
                                    j * HL * D + 128 * (p + 1)],
                        rhs=xs(ns, c),
                        start=(c == 0), stop=(c == CT - 1),
                    )
                dst = qk_t[(nm, p)][:, QW * ns:QW * (ns + 1)]
                # alternate PSUM evacuation between ScalarE and the DVE
                if qk_evac[0] % 2 == 0:
                    nc.scalar.copy(out=dst, in_=ps)
                else:
                    nc.vector.tensor_copy(out=dst, in_=ps)
                qk_evac[0] += 1

            # emission grouped by which x column block each matmul needs
            for t in range(4):
                vchunk(t)
            qkchunk("k", 0, 0), qkchunk("k", 1, 0)
            for t in range(4, 8):
                vchunk(t)
            qkchunk("k", 0, 1), qkchunk("k", 1, 1)
            qkchunk("q", 0, 0), qkchunk("q", 1, 0)
            for t in range(8, 12):
                vchunk(t)
            qkchunk("k", 0, 2), qkchunk("k", 1, 2)
            qkchunk("q", 0, 1), qkchunk("q", 1, 1)
            for t in range(12, 16):
                vchunk(t)
            qkchunk("k", 0, 3), qkchunk("k", 1, 3)
            for ns in (2, 3):
                qkchunk("q", 0, ns), qkchunk("q", 1, ns)

            phase1.close()
            # ---- phase 2 pools (reuse the x/w SBUF + psA PSUM space) --------
            psO = ctx.enter_context(tc.tile_pool(name="psO", bufs=4, space="PSUM"))
            otpool = ctx.enter_context(tc.tile_pool(name="ot", bufs=1))
            ptpool = ctx.enter_context(tc.tile_pool(name="pt", bufs=PEND))
            rpool = ctx.enter_context(tc.tile_pool(name="r", bufs=2))
            obpool = ctx.enter_context(tc.tile_pool(name="ob", bufs=4))

            out_qs = [nc.sync, nc.gpsimd]
            _out_rr = [0]

            def proj_chunk(qs, last=False):
                # PSUM -> fp16 SBUF staging split across ScalarE/DVE, then out.
                # Output DMAs round-robin over four issue queues so the final
                # chunk's descriptor-issue time does not serialize the drain.
                for t in range(QW // 128 * qs, QW // 128 * (qs + 1)):
                    ob = obpool.tile([128, C], F16, name="ob", tag="ob")
                    for nh in range(C // QW):
                        ps = psO.tile([128, QW], F32, name="psp", tag="ot")
                        for p2 in range(PAIRS):
                            nc.tensor.matmul(
                                ps,
                                lhsT=ot_sb[p2][qs][:, 128 * (t % (QW // 128)):
                                                   128 * (t % (QW // 128) + 1)],
                                rhs=wp_all[:, p2, QW * nh:QW * (nh + 1)],
                                start=(p2 == 0), stop=(p2 == PAIRS - 1),
                            )
                        dst = ob[:, QW * nh:QW * (nh + 1)]
                        if last:
                            # engine-parallel half-copies: the final chunk's
                            # staging sits on the strictly serial endgame
                            # path and both engines are otherwise idling
                            h = QW // 2
                            nc.scalar.copy(out=dst[:, 0:h], in_=ps[:, 0:h])
                            nc.vector.tensor_copy(out=dst[:, h:QW],
                                                  in_=ps[:, h:QW])
                        elif nh == 0:
                            nc.scalar.copy(out=dst, in_=ps)
                        else:
                            nc.vector.tensor_copy(out=dst, in_=ps)
                        orow = out[128 * t:128 * (t + 1), :]
                        eng = out_qs[_out_rr[0] % len(out_qs)]
                        _out_rr[0] += 1
                        eng.dma_start(
                            out=orow[:, QW * nh:QW * (nh + 1)], in_=dst)

            # ---- attention --------------------------------------------------
            ot_sb = [[otpool.tile([128, QW], BF16, name=f"otp{p}q{q}",
                                  tag=f"otp{p}q{q}")
                      for q in range(QS)] for p in range(PAIRS)]

            def emit_norm(pp, pqs, ot_a, ot_b, fast):
                # Normalization WITHOUT GpSimd partition_broadcast (that op
                # showed a constant ~4.5us queue wait gating every pair
                # boundary). The reciprocal row is broadcast across
                # partitions by a PE outer-product (ones[1,64].T @ rsb) into
                # the unused partitions 64-127 / vacated 0-63 of the ot_a
                # PSUM bank; the muls run on the DVE reading PSUM directly.
                # fast=True (last pair): den rows hop to partition 0 via DVE
                # cross-quadrant copies; else ScalarE stages them at
                # partition 64 and one SBUF-SBUF DMA moves them to 0.
                den0 = rpool.tile([1, 2 * QW + 8], F32, name="den0", tag="den0")
                rsb = rpool.tile([1, 2 * QW], F32, name="rsb", tag="rsb")
                rsbh = rpool.tile([1, 2 * QW], BF16, name="rsbh", tag="rsbh")
                rbc = rpool.tile([128, QW], F32, name="rbc", tag="rbc")
                dst = ot_sb[pp][pqs]

                def s_dummy(st):
                    # real dependency on the NEXT pair's k-block: keeps the
                    # chain's DVE ops out of the queue until mid-pair
                    nc.vector.tensor_copy(
                        out=den0[0:1, 2 * QW:2 * QW + 1], in_=st[0:1, 0:1])

                def s_den_a(st):
                    nc.scalar.copy(out=den0[0:1, 0:QW],
                                   in_=ot_a[64:65, :])

                def s_den_b(st):
                    nc.scalar.copy(out=den0[0:1, QW:2 * QW],
                                   in_=ot_b[64:65, :])

                def s_recip(st):
                    nc.vector.reciprocal_approx_fast(
                        out=rsb, in_=den0[0:1, 0:2 * QW])

                def s_cast(st):
                    # downcast the recip row (split over ScalarE/DVE) so the
                    # PE outer-products stream at bf16 rate (fp32 rhs is 2x
                    # slower); ~0.4% scale error, inside tolerance
                    nc.scalar.copy(out=rsbh[0:1, 0:QW], in_=rsb[0:1, 0:QW])
                    nc.vector.tensor_copy(out=rsbh[0:1, QW:2 * QW],
                                          in_=rsb[0:1, QW:2 * QW])

                def s_bcast(st):
                    # both recip rows broadcast by PE outer-products into the
                    # unused ot_a partitions 64..127 (A) and vacated den row
                    # region of ot_b (B: partitions 65..127 are free there)
                    nc.tensor.matmul(ot_a[64:128, :], lhsT=ones_rh,
                                     rhs=rsbh[0:1, 0:QW],
                                     start=True, stop=True)
                    nc.tensor.matmul(ot_b[64:128, :], lhsT=ones_rh,
                                     rhs=rsbh[0:1, QW:2 * QW],
                                     start=True, stop=True)

                def s_evac_a(st):
                    # ScalarE stages the broadcast rows to SBUF (lane-locked
                    # partition-preserving copies) - DVE ops may read only one
                    # PSUM operand
                    nc.scalar.copy(out=rbc[64:128, :], in_=ot_a[64:128, :])

                def s_evac_b(st):
                    nc.scalar.copy(out=rbc[0:64, :], in_=ot_b[64:128, :])

                def s_mul_a(st):
                    # 32-wide DVE ops: quadrant-aligned src windows may route
                    # to any output quadrant
                    nc.vector.tensor_mul(out=dst[0:32, :],
                                         in0=ot_a[0:32, :],
                                         in1=rbc[64:96, :])
                    nc.vector.tensor_mul(out=dst[32:64, :],
                                         in0=ot_a[32:64, :],
                                         in1=rbc[96:128, :])

                def s_mul_b(st):
                    # 64-wide quadrant-pair-aligned op crossing into 64..127
                    nc.vector.tensor_mul(out=dst[64:128, :],
                                         in0=ot_b[0:64, :],
                                         in1=rbc[0:64, :])

                steps = [s_den_a, s_den_b, s_recip, s_cast,
                         s_bcast, s_evac_a, s_evac_b, s_mul_a, s_mul_b]
                if not fast:
                    steps = [s_dummy] + steps
                return steps, den0, rsb

            # kb slots (in the NEXT pair) at which each deferred step emits:
            # late enough that by the time the DVE reaches the recip/mul in
            # its queue, the serial cross-engine chain has already completed
            NORM_SLOTS = (6, 7, 8, 9, 10, 11, 12, 13, 14, 15)
            norm_steps = []
            for qs in range(QS):
                for p in range(PAIRS):
                    kT = qk_t[("k", p)]
                    qT = qk_t[("q", p)]
                    last_pair = (qs == QS - 1 and p == PAIRS - 1)
                    ot_a = psO.tile([128, QW], F32, name="ot_a", tag="ot")
                    ot_b = psO.tile([128, QW], F32, name="ot_b", tag="ot")
                    pend = {}
                    for kb in range(TOK + PEND):
                        if kb < TOK:
                            st = psS.tile([128, 2 * QW], F32,
                                          name="st", tag="st")
                            st_a = st[:, 0:QW]
                            st_b = st[:, QW:2 * QW]
                            # scores for both heads into one PSUM tile; K=64
                            # row tiling runs the two matmuls concurrently
                            nc.tensor.matmul(
                                st_a,
                                lhsT=kT[0:64, 128 * kb:128 * (kb + 1)],
                                rhs=qT[0:64, QW * qs:QW * (qs + 1)],
                                start=True, stop=True,
                            )
                            nc.tensor.matmul(
                                st_b,
                                lhsT=kT[64:128, 128 * kb:128 * (kb + 1)],
                                rhs=qT[64:128, QW * qs:QW * (qs + 1)],
                                start=True, stop=True,
                            )
                            # one full-width exp per k-block, alternating
                            # engines (exact table exp on ScalarE /
                            # Schraudolph on the DVE): halves per-instruction
                            # overhead vs per-head splits. Tiles are written
                            # in their native dtype, READ through bitcast
                            if kb % 2 == 0:
                                pt_e = ptpool.tile([128, 2 * QW], BF16,
                                                   name="pt_e", tag="pte")
                                nc.scalar.activation(
                                    out=pt_e, in_=st, func=EXP, scale=0.125)
                                pend[kb] = (pt_e[:, 0:QW], pt_e[:, QW:2 * QW])
                            else:
                                pt_v = ptpool.tile([128, 2 * QW], I16,
                                                   name="pt_v", tag="ptv")
                                nc.vector.tensor_scalar(
                                    out=pt_v, in0=st,
                                    scalar1=EXPA, scalar2=EXPB,
                                    op0=MUL, op1=ADD)
                                pvb = pt_v.bitcast(BF16)
                                pend[kb] = (pvb[:, 0:QW], pvb[:, QW:2 * QW])
                            if norm_steps and kb in NORM_SLOTS:
                                norm_steps[NORM_SLOTS.index(kb)](st)
                                if kb == NORM_SLOTS[-1]:
                                    norm_steps = []
                        if kb >= PEND:
                            kv = kb - PEND
                            pa, pb = pend.pop(kv)
                            # O accumulation (64 V cols + ones col per head)
                            nc.tensor.matmul(
                                ot_a[0:65, :],
                                lhsT=v_t[kv][:, 2 * p, :],
                                rhs=pa,
                                start=(kv == 0), stop=(kv == TOK - 1),
                            )
                            nc.tensor.matmul(
                                ot_b[0:65, :],
                                lhsT=v_t[kv][:, 2 * p + 1, :],
                                rhs=pb,
                                start=(kv == 0), stop=(kv == TOK - 1),
                            )
                    # ---- normalization ---------------------------------
                    if last_pair:
                        # emitted immediately, per-head pipelined chains so
                        # the final projection's second accumulation half can
                        # start as soon as possible (this latency is the
                        # kernel's exposed tail)
                        den0 = rpool.tile([1, 2 * QW], F32,
                                          name="den0", tag="den0")
                        rsb = rpool.tile([1, 2 * QW], F32,
                                         name="rsb", tag="rsb")
                        rbc = rpool.tile([128, QW], F32, name="rbc", tag="rbc")
                        dst = ot_sb[p][qs]
                        # head A chain (den hop on the idle ScalarE -
                        # cross-partition scalar copies are proven legal)
                        nc.scalar.copy(out=den0[0:1, 0:QW],
                                       in_=ot_a[64:65, :])
                        nc.vector.reciprocal_approx_fast(
                            out=rsb[0:1, 0:QW], in_=den0[0:1, 0:QW])
                        nc.tensor.matmul(ot_a[64:128, :], lhsT=ones_r,
                                         rhs=rsb[0:1, 0:QW],
                                         start=True, stop=True)
                        nc.scalar.copy(out=rbc[64:128, :], in_=ot_a[64:128, :])
                        nc.vector.tensor_mul(out=dst[0:32, :],
                                             in0=ot_a[0:32, :],
                                             in1=rbc[64:96, :])
                        nc.vector.tensor_mul(out=dst[32:64, :],
                                             in0=ot_a[32:64, :],
                                             in1=rbc[96:128, :])
                        # head B chain
                        nc.scalar.copy(out=den0[0:1, QW:2 * QW],
                                       in_=ot_b[64:65, :])
                        nc.vector.reciprocal_approx_fast(
                            out=rsb[0:1, QW:2 * QW], in_=den0[0:1, QW:2 * QW])
                        if KEEPWARM:
                            kw = psS.tile([128, 2 * QW], F32,
                                          name="kw", tag="st")
                            nc.tensor.matmul(kw[0:64, 0:64],
                                             lhsT=den0[0:1, 0:64],
                                             rhs=den0[0:1, QW:QW + 64],
                                             start=True, stop=True)
                        nc.tensor.matmul(ot_b[64:128, :], lhsT=ones_r,
                                         rhs=rsb[0:1, QW:2 * QW],
                                         start=True, stop=True)
                        nc.scalar.copy(out=rbc[0:64, :], in_=ot_b[64:128, :])
                        nc.vector.tensor_mul(out=dst[64:128, :],
                                             in0=ot_b[0:64, :],
                                             in1=rbc[0:64, :])
                    else:
                        assert not norm_steps
                        norm_steps, _, _ = emit_norm(p, qs, ot_a, ot_b,
                                                     fast=False)

                # next query chunk's output projection (inputs long since
                # ready -> no PE stall)
                if qs > 0:
                    proj_chunk(qs - 1)
            proj_chunk(QS - 1, last=True)

    nc.compile()
    return nc


def _get_nc():
    if "nc" not in _cache:
        _cache["nc"] = _build_nc()
    return _cache["nc"]


def kernel(x, w_qkv, w_proj, b_proj):
    import ml_dtypes
    from concourse.bass_utils import run_bass_kernel_spmd

    x = np.asarray(x, dtype=np.float32)
    w_qkv = np.asarray(w_qkv, dtype=np.float32)
    w_proj = np.asarray(w_proj, dtype=np.float32)
    b_proj = np.asarray(b_proj, dtype=np.float32)

    nc = _get_nc()
    in_maps = []
    for core in range(NCORES):
        b, g = divmod(core, 4)
        rows = np.concatenate([
            np.arange(C * j + HL * D * g, C * j + HL * D * (g + 1))
            for j in range(3)
        ])
        wT = np.ascontiguousarray(w_qkv[rows].T)          # [1024, 768]
        xT = np.ascontiguousarray(x[b].T)                 # [1024, 2048]
        wp = np.ascontiguousarray(
            w_proj[:, HL * D * g:HL * D * (g + 1)].T)     # [256, 1024]
        x3 = xT.reshape(CT, 128, L).transpose(1, 0, 2)
        m = {
            f"xTb{i}": np.ascontiguousarray(
                x3[:, :, QW * i:QW * (i + 1)]).astype(ml_dtypes.bfloat16)
            for i in range(1, 4)
        }
        m["xTb0a"] = np.ascontiguousarray(
            x3[:, 0:4, 0:QW]).astype(ml_dtypes.bfloat16)
        m["xTb0b"] = np.ascontiguousarray(
            x3[:, 4:8, 0:QW]).astype(ml_dtypes.bfloat16)
        m.update({
            "wqkT": np.ascontiguousarray(
                wT[:, 0:2 * HL * D].reshape(CT, 128, 2 * HL * D)
                .transpose(1, 0, 2)
            ).astype(ml_dtypes.bfloat16),
            "wvT": np.ascontiguousarray(
                wT[:, 2 * HL * D:3 * HL * D].reshape(CT, 128, HL * D)
                .transpose(1, 0, 2)
            ).astype(ml_dtypes.bfloat16),
            "wpT": np.ascontiguousarray(
                wp.reshape(PAIRS, 128, C).transpose(1, 0, 2)
            ).astype(ml_dtypes.bfloat16),
        })
        in_maps.append(m)

    res = run_bass_kernel_spmd(
        nc, in_maps, list(range(NCORES)),
        trace=bool(os.environ.get("KERNEL_TRACE")),
    )
    _cache["last_results"] = res

    out = np.empty((B, L, C), dtype=np.float32)
    for b in range(B):
        acc = res.results[4 * b]["out"].astype(np.float32)
        for g in range(1, 4):
            acc = acc + res.results[4 * b + g]["out"]
        out[b] = acc + b_proj[None, :]
    return out
